# revision 1
# baseline (speedup 1.0000x reference)
"""BiModal attention kernel for Trainium2 (8 NeuronCores, data-parallel over batch).

Per core (one batch b): x, y: [2048, 128] fp32.
  S = x @ y.T                    (float32r matmuls, [2048, 2048])
  E = exp(S)                     (unshifted; softmax is shift-invariant and
                                  |S| <~ 67 so exp stays in fp32/bf16 range)
  a1 = (E @ y) / rowsum(E) * x
  a2 = (E.T @ x) / colsum(E) * y
  out = concat([a1, a2], -1)     ([2048, 256])

Layout: rows are relabeled r = 16*p + b (p = SBUF partition, b = block index)
so every DRAM transfer is contiguous per partition; the relabeling is applied
consistently to s and t everywhere, so the math is unchanged.

x^T / y^T (d-major, needed as f32r matmul operands) are built without the
tensor engine: split into bf16 hi/lo pairs (DVE/ACT), transpose both with the
DMA xbar, and re-merge hi+lo into f32r on GpSimd. bf16(hi)+bf16(lo) carries
~16 mantissa bits >= f32r's ~12, so S keeps f32r accuracy.

Main loop: two 1024-wide column panels; per (row block i): S matmuls (f32r)
-> exp (ACT, PSUM->SBUF bf16, fused row-sum accum) -> xbar transpose of E
into ET -> DVE column-sum partials. o1T chunks (contract over t) interleave
one panel behind to keep PE dense; o2T chunks + o1T tail + epilogues
(PE retranspose + fused gate on DVE) finish.
"""
import sys

sys.path.insert(0, "/opt/trn_rl_repo")

import os
import numpy as np

import concourse.bass as bass
import concourse.mybir as mybir
import concourse.tile as tile
from concourse.tile_rust import add_dep_helper
from concourse import bacc
from concourse.bass_utils import run_bass_kernel_spmd
from concourse.masks import make_identity

f32 = mybir.dt.float32
f32r = mybir.dt.float32r
bf16 = mybir.dt.bfloat16

B = 8
S = 2048
D = 128
P = 128
NB = S // P          # 16 blocks
NP = 2               # panels
PW = S // NP         # panel width (1024)
PB = PW // P         # blocks per panel (8)

_NC_CACHE = None
LAST_EXEC_NS = None


def _build_program(nc):
    x_d = nc.dram_tensor("x", [S, D], f32, kind="ExternalInput").ap()
    y_d = nc.dram_tensor("y", [S, D], f32, kind="ExternalInput").ap()
    out_d = nc.dram_tensor("out", [S, 2 * D], f32, kind="ExternalOutput").ap()

    # contiguous-per-partition views; row r = 16*p + b
    x_dv = x_d.rearrange("(p b) d -> p b d", p=P)      # [128, 16, 128]
    y_dv = y_d.rearrange("(p b) d -> p b d", p=P)
    out_dv = out_d.rearrange("(p b) c -> p b c", p=P)  # [128, 16, 256]

    Exp = mybir.ActivationFunctionType.Exp
    MUL = mybir.AluOpType.mult
    ADD = mybir.AluOpType.add
    SUBR = mybir.AluOpType.subtract
    AX = mybir.AxisListType.X

    with tile.TileContext(nc) as tc:
        with (
            tc.tile_pool(name="sb", bufs=1) as sb,
            tc.tile_pool(name="tp2", bufs=2) as tp2,
            tc.tile_pool(name="tpf", bufs=2) as tpf,
            tc.tile_pool(name="stg", bufs=6) as stg,
            tc.tile_pool(name="ps", bufs=1, space="PSUM") as ps,
        ):
            # ---- persistent SBUF tensors ----
            y_sb = tpf.tile([P, NB, D], f32, tag="vf32")
            x_sb = tpf.tile([P, NB, D], f32, tag="vf32")
            x_hi = sb.tile([P, NB, D], bf16, tag="x_hi")   # doubles as bf16 x
            y_hi = sb.tile([P, NB, D], bf16, tag="y_hi")   # doubles as bf16 y
            x_lo = sb.tile([P, NB, D], bf16, tag="x_lo")
            y_lo = sb.tile([P, NB, D], bf16, tag="y_lo")
            xT = sb.tile([P, NB, P], f32r, tag="xT")       # [d, sb, sp]
            yT = sb.tile([P, NB, P], f32r, tag="yT")       # [d, tb, tp]
            E = sb.tile([P, NB, S], bf16, tag="E")         # [sp, sb, t-pos]
            ET = sb.tile([P, NB, S], bf16, tag="ET")       # [tp, tb, s-pos]
            oT_pool_a = sb.tile([P, S], f32, tag="oT", name="oT_a")
            o1T_sb = oT_pool_a                             # [d, s-pos]
            ident = sb.tile([P, P], f32, tag="ident")
            l1p = sb.tile([P, 2 * NB], f32, tag="l1p")     # [sp, 2*i+ct]
            l2p = sb.tile([P, NB, NB], f32, tag="l2p")     # [tp, tb, i]
            l1 = sb.tile([P, NB], f32, tag="l1")
            l2 = sb.tile([P, NB], f32, tag="l2")
            r1 = sb.tile([P, NB], f32, tag="r1")
            r2 = sb.tile([P, NB], f32, tag="r2")

            make_identity(nc, ident[:])
            nc.sync.dma_start(y_sb[:, 0:PB], y_dv[:, 0:PB])
            nc.sync.dma_start(x_sb[:], x_dv)
            nc.sync.dma_start(y_sb[:, PB:NB], y_dv[:, PB:NB])

            # ---- prologue: xT/yT via hi/lo bf16 split + xbar transpose ----
            # hi = bf16(v) on ACT; lo = bf16(v - hi) on DVE;
            # xbar: [p, (b d)] -> [d, b, p]; merge hi+lo -> f32r on GpSimd.
            def build_T(v_sb, v_hi, v_lo, vT, name, halves=(0, 1)):
                tT_hi = tp2.tile([P, NB, P], bf16, tag="tT_hi", name=f"th_{name}")
                tT_lo = tp2.tile([P, NB, P], bf16, tag="tT_lo", name=f"tl_{name}")
                for h in halves:
                    sl = slice(h * PB, (h + 1) * PB)
                    nc.scalar.copy(v_hi[:, sl], v_sb[:, sl])
                    nc.vector.tensor_tensor(v_lo[:, sl], v_sb[:, sl],
                                            v_hi[:, sl], op=SUBR)
                    nc.sync.dma_start_transpose(
                        tT_hi[:, sl, :], v_hi[:, sl].rearrange("p b d -> p (b d)"))
                    nc.sync.dma_start_transpose(
                        tT_lo[:, sl, :], v_lo[:, sl].rearrange("p b d -> p (b d)"))
                    # first merge on DVE (fast, idle at head), rest on GpSimd
                    eng = nc.vector if (h == halves[0]) else nc.gpsimd
                    m = eng.tensor_tensor(vT[:, sl, :], tT_hi[:, sl, :],
                                          tT_lo[:, sl, :], op=ADD)
                return m

            m_y0 = build_T(y_sb, y_hi, y_lo, yT, "y", halves=(0,))
            build_T(x_sb, x_hi, x_lo, xT, "x")
            build_T(y_sb, y_hi, y_lo, yT, "y2", halves=(1,))

            # ---- main: panels of 1024 columns ----
            s_rot_a = ps.tile([P, PW], f32, tag="A0", name="s_rot_a")
            s_rot_b = ps.tile([P, PW], f32, tag="A1", name="s_rot_b")
            s_rot = [s_rot_a, s_rot_b]                   # S psum, 2-deep rotation
            o1_ps = ps.tile([P, 4, 512], f32, tag="B")   # o1T accumulator

            def o1_chunk(tb, pin=None):
                for q in range(4):
                    mm = nc.tensor.matmul(o1_ps[:, q, :], y_hi[:, tb, :],
                                          ET[:, tb, q * 512:(q + 1) * 512],
                                          start=(tb == 0), stop=(tb == NB - 1))
                    if q == 0 and pin is not None:
                        add_dep_helper(mm.ins, pin.ins, sync=False,
                                       reason="keep chunk at its emission slot")

            # PE warm-up: dense dummy matmuls so HAM unthrottles before S(0);
            # gated on the first merged data so they run during the x prologue
            yh_f = y_hi[:].rearrange("p b d -> p (b d)")
            for w in range(16):
                wm = nc.tensor.matmul(s_rot[0][:, 0:256], y_hi[:, 0, :],
                                      yh_f[:, 0:256], start=True, stop=True)
                if w == 0:
                    add_dep_helper(wm.ins, m_y0.ins, sync=True,
                                   reason="warmup starts once merges begin")

            yT_f = yT[:].rearrange("p b d -> p (b d)")
            for ct in range(NP):
                c0 = ct * PW
                for i in range(NB):
                    xti = xT[:, i, :]
                    slot = s_rot[i % 2][:]
                    nc.tensor.matmul(slot[:, 0:512], xti,
                                     yT_f[:, c0:c0 + 512], start=True, stop=True)
                    sm = nc.tensor.matmul(slot[:, 512:1024], xti,
                                          yT_f[:, c0 + 512:c0 + 1024],
                                          start=True, stop=True)
                    # interleave one o1T chunk of the previous panel (lagged so
                    # the chunk's transposed inputs are ready when PE reaches it)
                    if ct > 0 and 3 <= i < 3 + PB:
                        o1_chunk((ct - 1) * PB + (i - 3), pin=sm)
                    nc.scalar.activation(E[:, i, c0:c0 + PW], slot, Exp,
                                         accum_out=l1p[:, 2 * i + ct:2 * i + ct + 1])
                    nc.sync.dma_start_transpose(
                        ET[:, ct * PB:(ct + 1) * PB, i * P:(i + 1) * P],
                        E[:, i, c0:c0 + PW])
                    nc.vector.tensor_reduce(
                        l2p[:, ct * PB:(ct + 1) * PB, i],
                        ET[:, ct * PB:(ct + 1) * PB, i * P:(i + 1) * P],
                        axis=AX, op=ADD)

            # ---- normalizers ----
            nc.vector.tensor_reduce(l1[:], l1p[:].rearrange("p (i c) -> p i c", c=2),
                                    axis=AX, op=ADD)
            nc.vector.reciprocal(r1[:], l1[:])
            nc.vector.tensor_reduce(l2[:], l2p[:], axis=AX, op=ADD)
            nc.vector.reciprocal(r2[:], l2[:])

            # ---- final phase: o2T + trailing o1T chunks + epilogue 1 ----
            x_sb2 = tpf.tile([P, NB, D], f32, tag="vf32")
            nc.sync.dma_start(x_sb2[:], x_dv)
            y_sb2 = tpf.tile([P, NB, D], f32, tag="vf32")
            nc.sync.dma_start(y_sb2[:], y_dv)

            o2_ps_a = ps.tile([P, 2, 512], f32, tag="A0")
            o2_ps_b = ps.tile([P, 2, 512], f32, tag="A1")
            o2_q = [o2_ps_a[:, 0, :], o2_ps_a[:, 1, :], o2_ps_b[:, 0, :], o2_ps_b[:, 1, :]]

            e1_ps = None

            def epi1_step(j, pin=None):
                st1 = stg.tile([P, D], f32, tag="st", name=f"st1_{j}")
                tr = nc.tensor.transpose(e1_ps[:, j % 4, 0:P],
                                         o1T_sb[:, j * P:(j + 1) * P], ident[:])
                if pin is not None:
                    add_dep_helper(tr.ins, pin.ins, sync=False,
                                   reason="keep epi1 at its emission slot")
                nc.vector.scalar_tensor_tensor(st1[:], e1_ps[:, j % 4, 0:P],
                                               r1[:, j:j + 1], x_sb2[:, j, :],
                                               op0=MUL, op1=MUL)
                nc.sync.dma_start(out_dv[:, j, 0:D], st1[:])

            LAG = 4
            for i in range(NB):
                for q in range(4):
                    om = nc.tensor.matmul(o2_q[q], x_hi[:, i, :],
                                          E[:, i, q * 512:(q + 1) * 512],
                                          start=(i == 0), stop=(i == NB - 1))
                if LAG <= i < LAG + PB:
                    o1_chunk((NP - 1) * PB + (i - LAG), pin=om)
                if i == LAG + PB - 1:
                    # all o1T chunks issued; drain accumulator and start epi-1
                    nc.scalar.copy(o1T_sb[:, 0:1024],
                                   o1_ps[:, 0:2].rearrange("p a b -> p (a b)"))
                    nc.scalar.copy(o1T_sb[:, 1024:2048],
                                   o1_ps[:, 2:4].rearrange("p a b -> p (a b)"))
                    e1_ps = ps.tile([P, 4, 512], f32, tag="B")
                if i >= LAG + PB:
                    for k in range(4):
                        epi1_step(4 * (i - LAG - PB) + k, pin=om if k == 0 else None)

            o2T_sb = sb.tile([P, S], f32, tag="oT", name="oT_b")
            nc.scalar.copy(o2T_sb[:, 0:1024], o2_ps_a[:].rearrange("p a b -> p (a b)"))
            nc.scalar.copy(o2T_sb[:, 1024:2048], o2_ps_b[:].rearrange("p a b -> p (a b)"))

            # ---- epilogue 2: a2 = o2 * y * r2 (staged into dead E space) ----
            e2_rot = [ps.tile([P, 512], f32, tag="A0", name="e2a"),
                      ps.tile([P, 512], f32, tag="A1", name="e2b")]
            for j in range(NB):
                st2 = stg.tile([P, D], f32, tag="st", name=f"st2_{j}")
                e2t = e2_rot[j % 2]
                nc.tensor.transpose(e2t[:, 0:P],
                                    o2T_sb[:, j * P:(j + 1) * P], ident[:])
                nc.vector.scalar_tensor_tensor(st2[:], e2t[:, 0:P],
                                               r2[:, j:j + 1], y_sb2[:, j, :],
                                               op0=MUL, op1=MUL)
                nc.sync.dma_start(out_dv[:, j, D:2 * D], st2[:])

    nc.compile()
    return nc


def _get_nc():
    global _NC_CACHE
    if _NC_CACHE is None:
        nc = bacc.Bacc("TRN2", target_bir_lowering=False, debug=False,
                       num_devices=B)
        _NC_CACHE = _build_program(nc)
    return _NC_CACHE


def kernel(x, y):
    global LAST_EXEC_NS
    nc = _get_nc()
    x = np.asarray(x, dtype=np.float32)
    y = np.asarray(y, dtype=np.float32)
    in_maps = [
        {"x": np.ascontiguousarray(x[b]), "y": np.ascontiguousarray(y[b])}
        for b in range(B)
    ]
    trace = bool(int(os.environ.get("KERNEL_TRACE", "0")))
    res = run_bass_kernel_spmd(nc, in_maps, list(range(B)), trace=trace)
    LAST_EXEC_NS = res.exec_time_ns
    return np.stack([res.results[b]["out"] for b in range(B)], axis=0)



# revision 2
# speedup vs baseline: 1.3431x; 1.3431x over previous
"""BiModal attention kernel for Trainium2 (8 NeuronCores, data-parallel over batch).

Per core (one batch b): x, y: [2048, 128] fp32.
  S = x @ y.T                    (f32r matmuls, [2048, 2048])
  E = exp(S)                     (unshifted; softmax is shift-invariant and
                                  |S| <~ 67 so exp stays in fp32/bf16 range)
  a1 = (E @ y) / rowsum(E) * x
  a2 = (E.T @ x) / colsum(E) * y
  out = concat([a1, a2], -1)     ([2048, 256])

Layout: rows are relabeled r = 16*p + b (p = SBUF partition, b = block index)
so every DRAM transfer is contiguous per partition; the relabeling is applied
consistently to s and t everywhere, so the math is unchanged.

Structure (v2, rewritten for pipeline density):
 - prologue: x,y loaded on the sync HWDGE ring; xT/yT built with PE
   transpose-mode (f32, ident) staged through PSUM, copied to SBUF f32r.
   Dummy f32 matmuls at t=0 warm the HAM clock gate so real matmuls run
   at 2.4 GHz.
 - main loop over 16 row blocks i: S(i) panels p0/p1 -> exp on ACT
   (1024-wide, fused rowsum accum) -> DMA-xbar transpose of E row-panel to
   ET (contiguous dst: ET layout [tp, i, tb, sp]) -> DVE colsum partials.
   PE fills exp's shadow with o2-first-half (t 0:1024, moving E) and, from
   i>=8, o1-first-half (s 0:1024, moving ET) accumulation chunks.
 - tail: o2 second half (moving E) + o1 second half (moving ET) dense on
   PE, accumulator drains on ACT, epilogue per 128-block: PE retranspose,
   DVE fused gate (o * r * input), 4-block batched stores split across the
   sync and scalar HWDGE rings.

PSUM: A,B = S panel slots (2 banks each); C = o1A accum; D = o2A accum.
Tail reuses A->o2B, B->o1B, C/D -> epilogue transpose slots.
"""
import sys

sys.path.insert(0, "/opt/trn_rl_repo")

import os
import numpy as np

import concourse.bass as bass
import concourse.mybir as mybir
import concourse.tile as tile
from concourse import bacc
from concourse.bass_utils import run_bass_kernel_spmd
from concourse.masks import make_identity

f32 = mybir.dt.float32
f32r = mybir.dt.float32r
bf16 = mybir.dt.bfloat16

B = 8
S = 2048
D = 128
P = 128
NB = S // P          # 16 row/col blocks
HW = 1024            # panel (half) width

_NC_CACHE = None
LAST_EXEC_NS = None


def _build_program(nc):
    x_d = nc.dram_tensor("x", [S, D], f32, kind="ExternalInput").ap()
    y_d = nc.dram_tensor("y", [S, D], f32, kind="ExternalInput").ap()
    out_d = nc.dram_tensor("out", [S, 2 * D], f32, kind="ExternalOutput").ap()

    # contiguous-per-partition views; row r = 16*p + b
    x_dv = x_d.rearrange("(p b) d -> p b d", p=P)      # [128, 16, 128]
    y_dv = y_d.rearrange("(p b) d -> p b d", p=P)
    out_dv = out_d.rearrange("(p b) c -> p b c", p=P)  # [128, 16, 256]

    Exp = mybir.ActivationFunctionType.Exp
    MUL = mybir.AluOpType.mult
    ADD = mybir.AluOpType.add
    AX = mybir.AxisListType.X

    with tile.TileContext(nc) as tc:
        with (
            tc.tile_pool(name="sb", bufs=1) as sb,
            tc.tile_pool(name="stg", bufs=4) as stg,
            tc.tile_pool(name="ps", bufs=1, space="PSUM") as ps,
        ):
            # ---- persistent SBUF tensors ----
            y_sb = sb.tile([P, NB, D], f32, tag="y_sb")
            x_sb = sb.tile([P, NB, D], f32, tag="x_sb")
            x_hi = sb.tile([P, NB, D], bf16, tag="x_hi")   # bf16 x (o2 stationary)
            y_hi = sb.tile([P, NB, D], bf16, tag="y_hi")   # bf16 y (o1 stationary)
            xT = sb.tile([P, NB, P], f32r, tag="xT")       # [d, sb, sp]
            yT = sb.tile([P, NB, P], f32r, tag="yT")       # [d, tb, tp]
            E = sb.tile([P, NB, S], bf16, tag="E")         # [sp, i, t-pos]
            ET = sb.tile([P, NB, NB, P], bf16, tag="ET")   # [tp, i, tb, sp]
            o1T_sb = sb.tile([P, S], f32, tag="o1T")       # [d, s-pos]
            o2T_sb = sb.tile([P, S], f32, tag="o2T")       # [d, t-pos]
            ident = sb.tile([P, P], f32, tag="ident")
            scr = sb.tile([P, 1], f32, tag="scr")
            l1p = sb.tile([P, 2 * NB], f32, tag="l1p")     # [sp, 2*i+ct]
            l2p = sb.tile([P, NB, NB], f32, tag="l2p")     # [tp, tb, i]
            l1 = sb.tile([P, NB], f32, tag="l1")
            l2 = sb.tile([P, NB], f32, tag="l2")
            r1 = sb.tile([P, NB], f32, tag="r1")
            r2 = sb.tile([P, NB], f32, tag="r2")

            # ---- PSUM ----
            slotA = ps.tile([P, HW], f32, tag="A", name="slotA")
            slotB = ps.tile([P, HW], f32, tag="B", name="slotB")
            prestg = ps.tile([P, 8, P], f32, tag="C", name="prestg")
            o2A = ps.tile([P, HW], f32, tag="D", name="o2A")

            make_identity(nc, ident[:])
            # preload ACT exp table off the critical path
            nc.scalar.activation(scr[:], ident[:, 0:1], Exp)

            # ---- loads (sync HWDGE ring, FIFO) ----
            nc.sync.dma_start(y_sb[:, 0:8], y_dv[:, 0:8])
            nc.sync.dma_start(y_sb[:, 8:16], y_dv[:, 8:16])
            nc.sync.dma_start(x_sb[:, 0:8], x_dv[:, 0:8])
            nc.sync.dma_start(x_sb[:, 8:16], x_dv[:, 8:16])

            # bf16 stationaries: y_hi on ACT, x_hi on DVE (emitted into each
            # engine's FIFO where it doesn't stall later work)
            nc.scalar.copy(y_hi[:], y_sb[:])

            # ---- HAM warmup: dense dummy matmuls from ~t=0 (f32, 128-wide) ----
            for _ in range(8):
                nc.tensor.matmul(slotA[:, 0:P], ident[:], ident[:],
                                 start=True, stop=True)

            # ---- prologue: xT/yT via PE transpose staged through PSUM C ----
            def tgroup(src, dst, g, cp_dve):
                for k in range(4):
                    b = 4 * g + k
                    nc.tensor.transpose(prestg[:, b % 8, :], src[:, b, :],
                                        ident[:])
                s0 = (4 * g) % 8
                if cp_dve:
                    nc.vector.tensor_scalar_add(dst[:, 4 * g:4 * g + 4, :],
                                                prestg[:, s0:s0 + 4, :], 0.0)
                else:
                    nc.scalar.copy(dst[:, 4 * g:4 * g + 4, :],
                                   prestg[:, s0:s0 + 4, :])

            # PE FIFO: yg0 yg1 xg0 | S(0,p0) | yg2 yg3 | S(0,p1) | xg1..3
            tgroup(y_sb, yT, 0, True)
            tgroup(y_sb, yT, 1, True)
            tgroup(x_sb, xT, 0, False)

            yT_f = yT[:].rearrange("p b d -> p (b d)")

            def s_mm(i, half, slot):
                c0 = half * HW
                nc.tensor.matmul(slot[:, 0:512], xT[:, i, :],
                                 yT_f[:, c0:c0 + 512], start=True, stop=True)
                nc.tensor.matmul(slot[:, 512:1024], xT[:, i, :],
                                 yT_f[:, c0 + 512:c0 + 1024],
                                 start=True, stop=True)

            s_mm(0, 0, slotA)
            tgroup(y_sb, yT, 2, True)
            tgroup(y_sb, yT, 3, True)
            s_mm(0, 1, slotB)
            tgroup(x_sb, xT, 1, False)
            tgroup(x_sb, xT, 2, False)
            tgroup(x_sb, xT, 3, False)
            nc.vector.tensor_scalar_add(x_hi[:], x_sb[:], 0.0)

            # o1A accum lives in C after the last prestg read
            o1A = ps.tile([P, HW], f32, tag="C", name="o1A")

            def o2_mm(j, dst, c0, start, stop):
                nc.tensor.matmul(dst[:, 0:512], x_hi[:, j, :],
                                 E[:, j, c0:c0 + 512], start=start, stop=stop)
                nc.tensor.matmul(dst[:, 512:1024], x_hi[:, j, :],
                                 E[:, j, c0 + 512:c0 + 1024],
                                 start=start, stop=stop)

            def o1_mm(tb, dst, i0, start, stop):
                nc.tensor.matmul(dst[:, 0:512], y_hi[:, tb, :],
                                 ET[:, i0:i0 + 4, tb, :], start=start, stop=stop)
                nc.tensor.matmul(dst[:, 512:1024], y_hi[:, tb, :],
                                 ET[:, i0 + 4:i0 + 8, tb, :],
                                 start=start, stop=stop)

            # ---- main loop over row blocks ----
            for i in range(NB):
                if i > 0:
                    s_mm(i, 0, slotA)
                    o2_mm(i - 1, o2A, 0, start=(i - 1 == 0), stop=False)
                    s_mm(i, 1, slotB)
                if i >= 8:
                    for tb in (2 * (i - 8), 2 * (i - 8) + 1):
                        o1_mm(tb, o1A, 0, start=(tb == 0), stop=(tb == 15))
                # ACT: exp per panel, fused rowsum accumulation
                nc.scalar.activation(E[:, i, 0:HW], slotA[:], Exp,
                                     accum_out=l1p[:, 2 * i:2 * i + 1])
                nc.scalar.activation(E[:, i, HW:S], slotB[:], Exp,
                                     accum_out=l1p[:, 2 * i + 1:2 * i + 2])
                # sync ring: transpose E panels into ET (contiguous dst)
                nc.sync.dma_start_transpose(ET[:, i, 0:8, :], E[:, i, 0:HW])
                nc.sync.dma_start_transpose(ET[:, i, 8:16, :], E[:, i, HW:S])
                # DVE: colsum partials over the s-cols that just landed
                nc.vector.tensor_reduce(l2p[:, 0:8, i], ET[:, i, 0:8, :],
                                        axis=AX, op=ADD)
                nc.vector.tensor_reduce(l2p[:, 8:16, i], ET[:, i, 8:16, :],
                                        axis=AX, op=ADD)
            o2_mm(15, o2A, 0, start=False, stop=True)

            # ---- normalizers ----
            nc.vector.tensor_reduce(l1[:], l1p[:].rearrange(
                "p (i c) -> p i c", c=2), axis=AX, op=ADD)
            nc.vector.reciprocal(r1[:], l1[:])
            nc.vector.tensor_reduce(l2[:], l2p[:], axis=AX, op=ADD)
            nc.vector.reciprocal(r2[:], l2[:])

            # ---- tail: second halves ----
            o2B = ps.tile([P, HW], f32, tag="A", name="o2B")
            o1B = ps.tile([P, HW], f32, tag="B", name="o1B")
            for i in range(NB):
                o2_mm(i, o2B, HW, start=(i == 0), stop=(i == 15))
            for tb in range(NB):
                o1_mm(tb, o1B, 8, start=(tb == 0), stop=(tb == 15))

            # accumulator drains (ACT FIFO, dependency order)
            nc.scalar.copy(o2T_sb[:, 0:HW], o2A[:])
            nc.scalar.copy(o1T_sb[:, 0:HW], o1A[:])
            nc.scalar.copy(o2T_sb[:, HW:S], o2B[:])
            nc.scalar.copy(o1T_sb[:, HW:S], o1B[:])

            epiC = ps.tile([P, 8, P], f32, tag="C", name="epiC")
            epiD = ps.tile([P, 8, P], f32, tag="D", name="epiD")

            stage = {}

            def epi_t(side, j):
                """PE transpose for output block j of side (1|2)."""
                psv = epiC if side == 1 else epiD
                oT = o1T_sb if side == 1 else o2T_sb
                nc.tensor.transpose(psv[:, j % 8, :], oT[:, j * P:(j + 1) * P],
                                    ident[:])

            def epi_v(side, j):
                """DVE gate for output block j; store when 4-group filled."""
                psv = epiC if side == 1 else epiD
                rv = r1 if side == 1 else r2
                gate = x_sb if side == 1 else y_sb
                g = j // 4
                if j % 4 == 0:
                    stage[(side, g)] = stg.tile([P, 4, D], f32, tag="st",
                                                name=f"st{side}_{g}")
                st = stage[(side, g)]
                nc.vector.scalar_tensor_tensor(st[:, j % 4, :], psv[:, j % 8, :],
                                               rv[:, j:j + 1], gate[:, j, :],
                                               op0=MUL, op1=MUL)
                if j % 4 == 3:
                    c0 = 0 if side == 1 else D
                    eng = nc.sync if side == 1 else nc.scalar
                    eng.dma_start(out_dv[:, 4 * g:4 * g + 4, c0:c0 + D],
                                  st[:])

            # PE FIFO: epi2 j0..7 (after o2A drain), epi1 j0..7 (after o1A
            # drain), epi2 j8..15 (after o2B), epi1 j8..15 (after o1B).
            # DVE/stores follow the same order.
            for j in range(8):
                epi_t(2, j)
                epi_v(2, j)
            for j in range(8):
                epi_t(1, j)
                epi_v(1, j)
            for j in range(8, 16):
                epi_t(2, j)
                epi_v(2, j)
            for j in range(8, 16):
                epi_t(1, j)
                epi_v(1, j)

    nc.compile()
    return nc


def _get_nc():
    global _NC_CACHE
    if _NC_CACHE is None:
        nc = bacc.Bacc("TRN2", target_bir_lowering=False, debug=False,
                       num_devices=B)
        _NC_CACHE = _build_program(nc)
    return _NC_CACHE


def kernel(x, y):
    global LAST_EXEC_NS
    nc = _get_nc()
    x = np.asarray(x, dtype=np.float32)
    y = np.asarray(y, dtype=np.float32)
    in_maps = [
        {"x": np.ascontiguousarray(x[b]), "y": np.ascontiguousarray(y[b])}
        for b in range(B)
    ]
    trace = bool(int(os.environ.get("KERNEL_TRACE", "0")))
    res = run_bass_kernel_spmd(nc, in_maps, list(range(B)), trace=trace)
    LAST_EXEC_NS = res.exec_time_ns
    return np.stack([res.results[b]["out"] for b in range(B)], axis=0)
